# revision 1
# baseline (speedup 1.0000x reference)
"""Multi-head attention layer (T=1024, B=8, D=1024, H=16) on 8 TRN2 NeuronCores.

Sharding: data-parallel over batch B=8 -- one batch element per core, no
collectives. Each core computes its full attention layer slice:
  q/k/v projections -> causal softmax attention -> output projection.

Compute in bf16 on the TensorEngine (f32 PSUM accumulation); exp on ScalarE
fused with the PSUM eviction; softmax normalization via a ones-matmul column
reduction + fast approximate reciprocal fused into the output eviction.

Layouts (per core, host-side pre-transposed so every DMA is contiguous):
  xf/kf/vf: [D, T]  (X^T etc)       wq/wk/wv: [D, HN]    wo: [HN, D]
  On chip: QT/KT as [hn, t] (pairs of heads per 128 partitions), V as [t, hn].
  Scores computed transposed: ST[s, t] = KT_h.T-slice @ QT_h, 2-head
  row-packed (K=64). attn@V and Z (softmax denominator) 2-head col-packed.
"""

import numpy as np
import ml_dtypes

import concourse.bass as bass  # noqa: F401  (registers engine builders)
import concourse.bacc as bacc
import concourse.tile as tile
import concourse.mybir as mybir
from concourse.bass_utils import run_bass_kernel_spmd

T, B, D, H = 1024, 8, 1024, 16
NH = D // H          # 64 per-head width
P = 128              # SBUF partitions
NPAIR = H // 2       # 8 head-pairs (2 heads per 128 psum partitions)
DC = D // P          # 8 contraction chunks (bf16)
TT = T // P          # 8 t-tiles of 128
NCHUNK = 512         # matmul moving free dim / psum bank width (f32)
TC = T // NCHUNK     # 2 t-chunks
BF16 = mybir.dt.bfloat16
F32 = mybir.dt.float32
Exp = mybir.ActivationFunctionType.Exp
Copy = mybir.ActivationFunctionType.Copy
SCALE = 0.125  # 1/sqrt(NH)

N_CORES = 8
BF16_NP = ml_dtypes.bfloat16


def build_body(nc, tc, d, reps=1, causal=True, no_bias=False):
    """Emit the kernel body. d: dict of dram tensor handles."""
    import contextlib

    with contextlib.ExitStack() as ctx:
        sb_in = ctx.enter_context(tc.tile_pool(name="sb_in", bufs=2))
        sb_w = ctx.enter_context(tc.tile_pool(name="sb_w", bufs=2))
        sb_qkv = ctx.enter_context(tc.tile_pool(name="sb_qkv", bufs=1))
        sb_vte = ctx.enter_context(tc.tile_pool(name="sb_vte", bufs=NPAIR))
        sb_small = ctx.enter_context(tc.tile_pool(name="sb_small", bufs=1))
        sb_e = ctx.enter_context(tc.tile_pool(name="sb_e", bufs=7))
        sb_z = ctx.enter_context(tc.tile_pool(name="sb_z", bufs=2))
        sb_o = ctx.enter_context(tc.tile_pool(name="sb_o", bufs=2))
        ps = ctx.enter_context(tc.tile_pool(name="ps", bufs=1, space="PSUM"))

        # constants (loaded once)
        tri_t = sb_small.tile([P, P], BF16, tag="tri")
        ones64_t = sb_small.tile([P, NH], BF16, tag="ones64")
        ones1_t = sb_small.tile([1, P], BF16, tag="ones1")
        bqc_t = sb_small.tile([P, NPAIR], F32, tag="bqc")  # bq as [hn%128, pair]
        bkc_t = sb_small.tile([P, NPAIR], F32, tag="bkc")
        bv_t = sb_small.tile([1, D], BF16, tag="bv")
        bo_t = sb_small.tile([1, D], BF16, tag="bo")
        nc.sync.dma_start(out=tri_t[:], in_=d["tri"][:, :])
        nc.sync.dma_start(out=ones64_t[:], in_=d["ones64"][:, :])
        nc.sync.dma_start(out=ones1_t[:], in_=d["ones1"][:, :])
        nc.sync.dma_start(out=bqc_t[:], in_=d["bqc"][:, :])
        nc.sync.dma_start(out=bkc_t[:], in_=d["bkc"][:, :])
        nc.sync.dma_start(out=bv_t[:], in_=d["bv"][:, :])
        nc.sync.dma_start(out=bo_t[:], in_=d["bo"][:, :])

        # prefill the ones halves of all vte rotation slots once; V
        # evictions only ever write the V halves, so these survive reuse
        for _s in range(NPAIR):
            vte_fill = sb_vte.tile([P, TT, 2, P], BF16, tag="vte")
            for i in range(TT):
                nc.vector.tensor_copy(vte_fill[:, i, 0, NH:P], ones64_t[:])
                nc.vector.tensor_copy(vte_fill[:, i, 1, 0:NH], ones64_t[:])

        for _ in range(reps):
            # ---- load inputs/weights (tag-shared slots rotate per phase) ----
            # per-chunk DMAs so the first matmuls only wait on their own chunk
            def load_mat(dram, tag):
                t_ = sb_in.tile([P, DC, T], BF16, tag=tag, bufs=1)
                src = dram.ap().rearrange("(c p) t -> p c t", p=P)
                for r in range(DC):
                    nc.sync.dma_start(out=t_[:, r, :], in_=src[:, r, :])
                return t_

            xf_t = load_mat(d["xf"], "xf")
            kf_t = load_mat(d["kf"], "kf")
            vf_t = load_mat(d["vf"], "vf")
            wq_t = sb_w.tile([P, DC, D], BF16, tag="wqk")
            wk_t = sb_w.tile([P, DC, D], BF16, tag="wqk")
            wv_t = sb_w.tile([P, DC, D], BF16, tag="wv", bufs=1)
            # wo rotates into wq's slot once the last QT matmul has consumed it
            wo_t = sb_w.tile([P, DC, D], BF16, tag="wqk")
            for name, t_ in (("wq", wq_t), ("wk", wk_t), ("wv", wv_t), ("wo", wo_t)):
                src = d[name].ap().rearrange("(c p) n -> p c n", p=P)
                for r in range(DC):
                    nc.sync.dma_start(out=t_[:, r, :], in_=src[:, r, :])

            qt = sb_qkv.tile([P, NPAIR, T], BF16, tag="qt")
            kt = sb_qkv.tile([P, NPAIR, T], BF16, tag="kt")
            ots = sb_qkv.tile([P, NPAIR, T], BF16, tag="ots")

            # ---- per head-pair: QT/KT/V projections + attention ----
            # Projections for pair q+1 are emitted as closures interleaved
            # one-per-j into pair q's attention loop: the ~9 projection
            # matmuls give the PE fill work while ScalarE runs exp, instead
            # of stalling at the attn@V matmul.
            # vte: per-pair [keys, key-tile, head, V(64)|ones(64)] so one
            # M=128 stationary [V_h | ones64] makes a single e12 stream
            # produce attn@V (rows 0:64) AND the softmax denominator
            # broadcast to rows 64:128 -- no standalone Z matmuls
            # re-streaming every e12 tile. One slot per pair (bufs=NPAIR)
            # so the batched V eviction never WARs a live pair.
            vte_list = [
                sb_vte.tile([P, TT, 2, P], BF16, tag="vte", name=f"vte{_q}")
                for _q in range(NPAIR)
            ]

            def proj_units(q):
                hn0 = q * P
                units = []
                for dst, w_t, in_t, b_t in (
                    (qt, wq_t, xf_t, bqc_t), (kt, wk_t, kf_t, bkc_t)
                ):
                    for c in range(TC):
                        def f_qk(dst=dst, w_t=w_t, in_t=in_t, b_t=b_t,
                                 c=c, hn0=hn0, q=q):
                            t0 = c * NCHUNK
                            pj = ps.tile([P, NCHUNK], F32, tag="mm4", bufs=2)
                            for r in range(DC):
                                nc.tensor.matmul(
                                    pj[:],
                                    w_t[:, r, hn0 : hn0 + P],
                                    in_t[:, r, t0 : t0 + NCHUNK],
                                    start=(r == 0),
                                    stop=(r == DC - 1),
                                )
                            if no_bias:
                                nc.vector.tensor_copy(
                                    dst[:, q, t0 : t0 + NCHUNK], pj[:])
                            else:
                                # bias folded into eviction (per-part scalar)
                                nc.vector.tensor_scalar_add(
                                    dst[:, q, t0 : t0 + NCHUNK], pj[:],
                                    b_t[:, q : q + 1])
                        units.append(f_qk)
                return units

            def vm_unit(half, i):
                # V projection, 4 head-pairs per N=512 stream:
                # out pj [t-tile, 512 hn] covers pairs 4*half..4*half+3
                def f(half=half, i=i):
                    tt0 = i * P
                    n0 = half * NCHUNK
                    pj = ps.tile([P, NCHUNK], F32, tag="mm4", bufs=2)
                    for r in range(DC):
                        nc.tensor.matmul(
                            pj[:],
                            vf_t[:, r, tt0 : tt0 + P],
                            wv_t[:, r, n0 : n0 + NCHUNK],
                            start=(r == 0),
                            stop=(r == DC - 1) and no_bias,
                        )
                    if not no_bias:
                        nc.tensor.matmul(
                            pj[:],
                            ones1_t[0:1, :],
                            bv_t[0:1, n0 : n0 + NCHUNK],
                            start=False,
                            stop=True,
                        )
                    # head1 slot: [V_h1 | ones] -> otz1 = [ot1; z1]
                    # head2 slot: [ones | V_h2] -> otz2 = [z2; ot2]
                    # (keeps every eviction tensor_tensor op partition-
                    # aligned; engines anchor reads at the OUT window)
                    for p4 in range(4):
                        vte = vte_list[half * 4 + p4]
                        c0 = p4 * P
                        nc.vector.tensor_copy(
                            vte[:, i, 0, 0:NH], pj[:, c0 : c0 + NH])
                        nc.vector.tensor_copy(
                            vte[:, i, 1, NH:P], pj[:, c0 + NH : c0 + P])
                return f

            def wo_unit(i, c2):
                def f(i=i, c2=c2):
                    tt0 = i * P
                    n0 = c2 * NCHUNK
                    po = ps.tile([P, NCHUNK], F32, tag="mm4", bufs=2)
                    for qq in range(NPAIR):
                        nc.tensor.matmul(
                            po[:],
                            ots[:, qq, tt0 : tt0 + P],
                            wo_t[:, qq, n0 : n0 + NCHUNK],
                            start=(qq == 0),
                            stop=(qq == NPAIR - 1) and no_bias,
                        )
                    if not no_bias:
                        nc.tensor.matmul(
                            po[:],
                            ones1_t[0:1, :],
                            bo_t[0:1, n0 : n0 + NCHUNK],
                            start=False,
                            stop=True,
                        )
                    so = sb_o.tile([P, NCHUNK], F32, tag="so")
                    nc.scalar.activation(so[:], po[:], Copy)
                    nc.sync.dma_start(
                        out=d["out"][tt0 : tt0 + P, n0 : n0 + NCHUNK], in_=so[:]
                    )
                return f

            wo_pending = [wo_unit(i, c2) for i in range(TT) for c2 in range(TC)]

            for f in proj_units(0):
                f()
            for i in range(TT):
                vm_unit(0, i)()
            for q in range(NPAIR):
                vte_cur = vte_list[q]
                if q == 0:
                    filler = proj_units(1) + [vm_unit(1, i) for i in range(TT)]
                elif q + 1 < NPAIR:
                    filler = proj_units(q + 1)
                else:
                    # last pair: fill its second t-chunk with the Wo units
                    # whose output t-tiles (< 512) only need ots chunks that
                    # are complete once this pair's first chunk finishes.
                    filler = [wo_pending.pop(0)
                              for _ in range(2 * TC * TC)]  # i 0..3, both c2

                # attention for this pair, scores pipelined one j ahead
                for c in range(TC):
                    t0 = c * NCHUNK
                    jmax = 4 * (c + 1) if causal else TT
                    otz1 = ps.tile([P, NCHUNK], F32, tag="ot", bufs=2)
                    otz2 = ps.tile([P, NCHUNK], F32, tag="ot", bufs=2)

                    def offn(j):
                        off = max(0, j * P - t0) if causal else 0
                        return off, NCHUNK - off

                    def compute_st(j):
                        off, n = offn(j)
                        s0 = j * P
                        st1 = ps.tile([P, NCHUNK], F32, tag="st", bufs=4)
                        st2 = ps.tile([P, NCHUNK], F32, tag="st", bufs=4)
                        nc.tensor.matmul(
                            st1[:, :n],
                            kt[0:64, q, s0 : s0 + P],
                            qt[0:64, q, t0 + off : t0 + NCHUNK],
                            start=True, stop=True, tile_position=(0, 0),
                        )
                        nc.tensor.matmul(
                            st2[:, :n],
                            kt[64:128, q, s0 : s0 + P],
                            qt[64:128, q, t0 + off : t0 + NCHUNK],
                            start=True, stop=True, tile_position=(64, 0),
                        )
                        return st1, st2

                    sts = [compute_st(0)]
                    for j in range(jmax):
                        off, n = offn(j)
                        s0 = j * P
                        first, last = (j == 0), (j == jmax - 1)
                        st1, st2 = sts[j]
                        if j + 1 < jmax:
                            sts.append(compute_st(j + 1))
                        if filler and (q + 1 < NPAIR or c == 1):
                            # next pair's projection unit (or, on the last
                            # pair's second chunk, a ready Wo unit)
                            filler.pop(0)()
                        e12 = sb_e.tile([P, 2 * NCHUNK], BF16, tag="e12")
                        nc.scalar.activation(
                            e12[:, :n], st1[:, :n], Exp, scale=SCALE)
                        nc.scalar.activation(
                            e12[:, NCHUNK : NCHUNK + n],
                            st2[:, :n], Exp, scale=SCALE)
                        if causal and s0 >= t0:
                            # diagonal tile: keep s <= t; multiplicative 0/1
                            # mask post-exp on the otherwise idle GpSimd
                            nc.gpsimd.tensor_mul(e12[:, 0:P], e12[:, 0:P], tri_t[:])
                            nc.gpsimd.tensor_mul(
                                e12[:, NCHUNK : NCHUNK + P],
                                e12[:, NCHUNK : NCHUNK + P], tri_t[:])
                        # attn @ V fused with Z: stationary [V_h | ones64]
                        # (M=128) -> one e12 stream writes attn@V to rows
                        # 0:64 and the softmax denominator broadcast to
                        # rows 64:128.
                        nc.tensor.matmul(
                            otz1[:, off:], vte_cur[:, j, 0, :],
                            e12[:, :n], start=first, stop=last,
                        )
                        nc.tensor.matmul(
                            otz2[:, off:], vte_cur[:, j, 1, :],
                            e12[:, NCHUNK : NCHUNK + n],
                            start=first, stop=last,
                        )
                    # cross-partition z moves must be plain copies (fused
                    # DVE ISA ops ignore the input partition offset on HW)
                    zs = sb_z.tile([P, NCHUNK], F32, tag="zs")
                    nc.scalar.activation(zs[0:NH, :], otz1[NH:P, :], Copy)
                    nc.vector.tensor_copy(zs[NH:P, :], otz2[0:NH, :])
                    zinv = sb_z.tile([P, NCHUNK], F32, tag="zinv")
                    nc.vector.reciprocal_approx_fast(out=zinv[:], in_=zs[:])
                    nc.vector.tensor_mul(
                        ots[0:NH, q, t0 : t0 + NCHUNK],
                        otz1[0:NH, :], zinv[0:NH, :])
                    nc.vector.tensor_mul(
                        ots[NH:P, q, t0 : t0 + NCHUNK],
                        otz2[NH:P, :], zinv[NH:P, :])
                for f in filler:  # leftovers (non-causal has more js than units)
                    f()

            # ---- output projection (tail: units not already interleaved) ----
            for f in wo_pending:
                f()


def build(causal=True, reps=1, no_bias=False):
    nc = bacc.Bacc("TRN2", target_bir_lowering=False, debug=False,
                   num_devices=N_CORES)
    d = {}
    for name in ("xf", "kf", "vf"):
        d[name] = nc.dram_tensor(name, [D, T], BF16, kind="ExternalInput")
    for name in ("wq", "wk", "wv", "wo"):
        d[name] = nc.dram_tensor(name, [D, D], BF16, kind="ExternalInput")
    for name in ("bv", "bo"):
        d[name] = nc.dram_tensor(name, [1, D], BF16, kind="ExternalInput")
    for name in ("bqc", "bkc"):
        d[name] = nc.dram_tensor(name, [P, NPAIR], F32, kind="ExternalInput")
    d["tri"] = nc.dram_tensor("tri", [P, P], BF16, kind="ExternalInput")
    d["ones64"] = nc.dram_tensor("ones64", [P, NH], BF16, kind="ExternalInput")
    d["ones1"] = nc.dram_tensor("ones1", [1, P], BF16, kind="ExternalInput")
    d["out"] = nc.dram_tensor("out", [T, D], F32, kind="ExternalOutput")

    with tile.TileContext(nc) as tc:
        build_body(nc, tc, d, reps=reps, causal=causal, no_bias=no_bias)
    nc.compile()
    return nc


def make_in_maps(input_tensor, keys_vector, values_vector, Wq, bq, Wk, bk,
                 Wv, bv, Wo, bo):
    """Host-side sharding + layout transforms + bf16 casts."""
    def b16(a):
        return np.ascontiguousarray(a).astype(BF16_NP)

    shared = {
        "wq": b16(np.asarray(Wq, np.float32).transpose(1, 0, 2).reshape(D, D)),
        "wk": b16(np.asarray(Wk, np.float32).transpose(1, 0, 2).reshape(D, D)),
        "wv": b16(np.asarray(Wv, np.float32).transpose(1, 0, 2).reshape(D, D)),
        "wo": b16(np.asarray(Wo, np.float32).T),
        # bq/bk as [hn % 128, pair] f32 columns for per-partition bias add
        "bqc": np.ascontiguousarray(
            np.asarray(bq, np.float32).reshape(NPAIR, P).T),
        "bkc": np.ascontiguousarray(
            np.asarray(bk, np.float32).reshape(NPAIR, P).T),
        "bv": b16(np.asarray(bv, np.float32).reshape(1, D)),
        "bo": b16(np.asarray(bo, np.float32).reshape(1, D)),
        "tri": np.triu(np.ones((P, P), np.float32)).astype(BF16_NP),
        "ones64": np.ones((P, NH), BF16_NP),
        "ones1": np.ones((1, P), BF16_NP),
    }
    x = np.asarray(input_tensor, np.float32)
    k = np.asarray(keys_vector, np.float32)
    v = np.asarray(values_vector, np.float32)
    in_maps = []
    for c in range(N_CORES):
        m = dict(shared)
        m["xf"] = b16(x[:, c, :].T)
        m["kf"] = b16(k[:, c, :].T)
        m["vf"] = b16(v[:, c, :].T)
        in_maps.append(m)
    return in_maps


_NC_CACHE = {}


def kernel(input_tensor, keys_vector, values_vector, Wq, bq, Wk, bk, Wv, bv,
           Wo, bo, mask):
    causal = bool(int(np.asarray(mask)))
    no_bias = all(
        not np.any(np.asarray(b)) for b in (bq, bk, bv, bo)
    )
    key = (causal, no_bias)
    if key not in _NC_CACHE:
        _NC_CACHE[key] = build(causal=causal, no_bias=no_bias)
    nc = _NC_CACHE[key]
    in_maps = make_in_maps(input_tensor, keys_vector, values_vector, Wq, bq,
                           Wk, bk, Wv, bv, Wo, bo)
    res = run_bass_kernel_spmd(nc, in_maps, core_ids=list(range(N_CORES)))
    out = np.empty((T, B, D), np.float32)
    for c in range(N_CORES):
        out[:, c, :] = res.results[c]["out"]
    return out



# revision 25
# speedup vs baseline: 3.7636x; 3.7636x over previous
"""Multi-head attention layer (T=1024, B=8, D=1024, H=16) on 8 TRN2 NeuronCores.

Sharding: data-parallel over batch B=8 -- one batch element per core, no
collectives. Each core computes its full attention layer slice:
  q/k/v projections -> causal softmax attention -> output projection.

Compute in bf16 on the TensorEngine (f32 PSUM accumulation); exp on ScalarE
fused with the PSUM eviction; softmax normalization via a ones-matmul column
reduction + fast approximate reciprocal fused into the output eviction.

Layouts (per core, host-side pre-transposed so every DMA is contiguous):
  xf/kf/vf: [D, T]  (X^T etc)       wq/wk/wv: [D, HN]    wo: [HN, D]
  On chip: QT/KT as [hn, t] (pairs of heads per 128 partitions), V as [t, hn].
  Scores computed transposed: ST[s, t] = KT_h.T-slice @ QT_h, 2-head
  row-packed (K=64). attn@V and Z (softmax denominator) 2-head col-packed.

Scheduling: the two per-head score tiles live in one 2-bank PSUM tile
[P, 2, NCHUNK] so a single strided-AP Exp activation evicts both; score
matmuls run two j-tiles ahead of the attn@V consumer so ScalarE latency
stays off the PE critical path; the projection/output matmul "filler"
units are paced evenly across all attention j-steps so the PE never idles
waiting on exp; the causal mask runs on the otherwise-idle GpSimd (an
empty queue -- DVE's FIFO backlog would delay the attn@V consumer); each
weight has its own SBUF slot so the next rep's weight DMAs only wait on
that weight's own last use; and the vte stationaries share one ones block
per pair ([V_h1 | ones | V_h2]) to fit everything in SBUF.
"""

import numpy as np
import ml_dtypes

import concourse.bass as bass  # noqa: F401  (registers engine builders)
import concourse.bacc as bacc
import concourse.tile as tile
import concourse.mybir as mybir
from concourse.bass_utils import run_bass_kernel_spmd

T, B, D, H = 1024, 8, 1024, 16
NH = D // H          # 64 per-head width
P = 128              # SBUF partitions
NPAIR = H // 2       # 8 head-pairs (2 heads per 128 psum partitions)
DC = D // P          # 8 contraction chunks (bf16)
TT = T // P          # 8 t-tiles of 128
NCHUNK = 512         # matmul moving free dim / psum bank width (f32)
TC = T // NCHUNK     # 2 t-chunks
BF16 = mybir.dt.bfloat16
F32 = mybir.dt.float32
Exp = mybir.ActivationFunctionType.Exp
Copy = mybir.ActivationFunctionType.Copy
SCALE = 0.125  # 1/sqrt(NH)

N_CORES = 8
BF16_NP = ml_dtypes.bfloat16


def build_body(nc, tc, d, reps=1, causal=True, no_bias=False):
    """Emit the kernel body. d: dict of dram tensor handles."""
    import contextlib

    with contextlib.ExitStack() as ctx:
        sb_in = ctx.enter_context(tc.tile_pool(name="sb_in", bufs=2))
        sb_w = ctx.enter_context(tc.tile_pool(name="sb_w", bufs=2))
        sb_qkv = ctx.enter_context(tc.tile_pool(name="sb_qkv", bufs=1))
        sb_vte = ctx.enter_context(tc.tile_pool(name="sb_vte", bufs=NPAIR))
        sb_small = ctx.enter_context(tc.tile_pool(name="sb_small", bufs=1))
        sb_e = ctx.enter_context(tc.tile_pool(name="sb_e", bufs=3))
        sb_z = ctx.enter_context(tc.tile_pool(name="sb_z", bufs=2))
        sb_o = ctx.enter_context(tc.tile_pool(name="sb_o", bufs=2))
        ps = ctx.enter_context(tc.tile_pool(name="ps", bufs=1, space="PSUM"))

        # constants (loaded once)
        tri2_t = sb_small.tile([P, 2, P], BF16, tag="tri2")  # tri twice
        ones64_t = sb_small.tile([P, NH], BF16, tag="ones64")
        ones1_t = sb_small.tile([1, P], BF16, tag="ones1")
        bqc_t = sb_small.tile([P, NPAIR], F32, tag="bqc")  # bq as [hn%128, pair]
        bkc_t = sb_small.tile([P, NPAIR], F32, tag="bkc")
        bv_t = sb_small.tile([1, D], BF16, tag="bv")
        bo_t = sb_small.tile([1, D], BF16, tag="bo")
        nc.sync.dma_start(out=tri2_t[:], in_=d["tri2"][:, :, :])
        nc.sync.dma_start(out=ones64_t[:], in_=d["ones64"][:, :])
        nc.sync.dma_start(out=ones1_t[:], in_=d["ones1"][:, :])
        nc.sync.dma_start(out=bqc_t[:], in_=d["bqc"][:, :])
        nc.sync.dma_start(out=bkc_t[:], in_=d["bkc"][:, :])
        nc.sync.dma_start(out=bv_t[:], in_=d["bv"][:, :])
        nc.sync.dma_start(out=bo_t[:], in_=d["bo"][:, :])

        # prefill the shared ones block of all vte rotation slots once; V
        # evictions only ever write the V thirds, so these survive reuse
        for _s in range(NPAIR):
            vte_fill = sb_vte.tile([P, TT, 3, NH], BF16, tag="vte")
            for i in range(TT):
                nc.vector.tensor_copy(vte_fill[:, i, 1, :], ones64_t[:])

        for _ in range(reps):
            # ---- load inputs/weights (tag-shared slots rotate per phase) ----
            # per-chunk DMAs so the first matmuls only wait on their own chunk
            def load_mat(dram, tag):
                t_ = sb_in.tile([P, DC, T], BF16, tag=tag, bufs=1)
                src = dram.ap().rearrange("(c p) t -> p c t", p=P)
                for r in range(DC):
                    nc.sync.dma_start(out=t_[:, r, :], in_=src[:, r, :])
                return t_

            xf_t = load_mat(d["xf"], "xf")
            kf_t = load_mat(d["kf"], "kf")
            vf_t = load_mat(d["vf"], "vf")
            # one slot per weight: rep r+1's DMA only waits on rep r's last
            # use of the SAME weight (all consumed well before rep end
            # except wo, whose reload has the whole next rep as slack)
            wq_t = sb_w.tile([P, DC, D], BF16, tag="wq", bufs=1)
            wk_t = sb_w.tile([P, DC, D], BF16, tag="wk", bufs=1)
            wv_t = sb_w.tile([P, DC, D], BF16, tag="wv", bufs=1)
            wo_t = sb_w.tile([P, DC, D], BF16, tag="wo", bufs=1)
            for name, t_ in (("wq", wq_t), ("wk", wk_t), ("wv", wv_t), ("wo", wo_t)):
                src = d[name].ap().rearrange("(c p) n -> p c n", p=P)
                for r in range(DC):
                    nc.sync.dma_start(out=t_[:, r, :], in_=src[:, r, :])

            qt = sb_qkv.tile([P, NPAIR, T], BF16, tag="qt")
            kt = sb_qkv.tile([P, NPAIR, T], BF16, tag="kt")
            ots = sb_qkv.tile([P, NPAIR, T], BF16, tag="ots")

            # ---- per head-pair: QT/KT/V projections + attention ----
            # Projection/output units are emitted as closures and paced
            # evenly across the attention j-steps: the ~9 matmuls per unit
            # give the PE fill work while ScalarE runs exp, instead of
            # stalling at the attn@V matmul.
            # vte: per-pair [keys, key-tile, V_h1|ones|V_h2] (the ones
            # block is shared between the pair's heads) so one M=128
            # stationary [V_h1|ones] (cols 0:128) or [ones|V_h2] (cols
            # 64:192) makes a single e12 stream produce attn@V on one
            # 64-row half AND the softmax denominator broadcast on the
            # other -- no standalone Z matmuls. One slot per pair
            # (bufs=NPAIR) so the batched V eviction never WARs a live
            # pair.
            vte_list = [
                sb_vte.tile([P, TT, 3, NH], BF16, tag="vte", name=f"vte{_q}")
                for _q in range(NPAIR)
            ]

            def proj_units(q):
                hn0 = q * P
                units = []
                for dst, w_t, in_t, b_t in (
                    (qt, wq_t, xf_t, bqc_t), (kt, wk_t, kf_t, bkc_t)
                ):
                    for c in range(TC):
                        def f_qk(dst=dst, w_t=w_t, in_t=in_t, b_t=b_t,
                                 c=c, hn0=hn0, q=q):
                            t0 = c * NCHUNK
                            pj = ps.tile([P, NCHUNK], F32, tag="mm4", bufs=2)
                            for r in range(DC):
                                nc.tensor.matmul(
                                    pj[:],
                                    w_t[:, r, hn0 : hn0 + P],
                                    in_t[:, r, t0 : t0 + NCHUNK],
                                    start=(r == 0),
                                    stop=(r == DC - 1),
                                )
                            if no_bias:
                                nc.vector.tensor_copy(
                                    dst[:, q, t0 : t0 + NCHUNK], pj[:])
                            else:
                                # bias folded into eviction (per-part scalar)
                                nc.vector.tensor_scalar_add(
                                    dst[:, q, t0 : t0 + NCHUNK], pj[:],
                                    b_t[:, q : q + 1])
                        units.append(f_qk)
                return units

            def vm_unit(half, i):
                # V projection, 4 head-pairs per N=512 stream:
                # out pj [t-tile, 512 hn] covers pairs 4*half..4*half+3
                def f(half=half, i=i):
                    tt0 = i * P
                    n0 = half * NCHUNK
                    pj = ps.tile([P, NCHUNK], F32, tag="mm4", bufs=2)
                    for r in range(DC):
                        nc.tensor.matmul(
                            pj[:],
                            vf_t[:, r, tt0 : tt0 + P],
                            wv_t[:, r, n0 : n0 + NCHUNK],
                            start=(r == 0),
                            stop=(r == DC - 1) and no_bias,
                        )
                    if not no_bias:
                        nc.tensor.matmul(
                            pj[:],
                            ones1_t[0:1, :],
                            bv_t[0:1, n0 : n0 + NCHUNK],
                            start=False,
                            stop=True,
                        )
                    # head1 slot: [V_h1 | ones] -> otz1 = [ot1; z1]
                    # head2 slot: [ones | V_h2] -> otz2 = [z2; ot2]
                    # (keeps every eviction tensor_tensor op partition-
                    # aligned; engines anchor reads at the OUT window)
                    for p4 in range(4):
                        vte = vte_list[half * 4 + p4]
                        c0 = p4 * P
                        nc.vector.tensor_copy(
                            vte[:, i, 0, :], pj[:, c0 : c0 + NH])
                        nc.vector.tensor_copy(
                            vte[:, i, 2, :], pj[:, c0 + NH : c0 + P])
                return f

            def wo_unit(i, c2):
                def f(i=i, c2=c2):
                    tt0 = i * P
                    n0 = c2 * NCHUNK
                    po = ps.tile([P, NCHUNK], F32, tag="mm4", bufs=2)
                    for qq in range(NPAIR):
                        nc.tensor.matmul(
                            po[:],
                            ots[:, qq, tt0 : tt0 + P],
                            wo_t[:, qq, n0 : n0 + NCHUNK],
                            start=(qq == 0),
                            stop=(qq == NPAIR - 1) and no_bias,
                        )
                    if not no_bias:
                        nc.tensor.matmul(
                            po[:],
                            ones1_t[0:1, :],
                            bo_t[0:1, n0 : n0 + NCHUNK],
                            start=False,
                            stop=True,
                        )
                    # bf16 eviction: half the SBUF bounce + half the DMA
                    # bytes; host upconverts the final output to f32
                    so = sb_o.tile([P, NCHUNK], BF16, tag="so")
                    nc.scalar.activation(so[:], po[:], Copy)
                    nc.sync.dma_start(
                        out=d["out"][tt0 : tt0 + P, n0 : n0 + NCHUNK], in_=so[:]
                    )
                return f

            wo_pending = [wo_unit(i, c2) for i in range(TT) for c2 in range(TC)]

            # upfront: only what pair 0 chunk 0 strictly needs (its own q/k
            # projections and the first half of its vte tiles); the rest is
            # paced into the attention j-steps below
            for f in proj_units(0):
                f()
            for i in range(4):
                vm_unit(0, i)()

            # filler availability per pair: vm units first (earliest
            # deadline), then the next pair's q/k projections
            fill_plan = {
                0: [vm_unit(0, i) for i in range(4, TT)] + proj_units(1),
                1: [vm_unit(1, 0), vm_unit(1, 1)] + proj_units(2),
                2: [vm_unit(1, 2), vm_unit(1, 3)] + proj_units(3),
                3: [vm_unit(1, 4), vm_unit(1, 5)] + proj_units(4),
                4: [vm_unit(1, 6), vm_unit(1, 7)] + proj_units(5),
                5: proj_units(6),
                6: proj_units(7),
                7: [],
            }

            for q in range(NPAIR):
                vte_cur = vte_list[q]
                filler = fill_plan[q]
                pair_steps = (4 + 8) if causal else 2 * TT
                nfill = len(filler)
                spent = 0
                step = 0

                # attention for this pair; scores run two j-tiles ahead of
                # the attn@V consumer so exp latency is off the PE path
                for c in range(TC):
                    t0 = c * NCHUNK
                    jmax = 4 * (c + 1) if causal else TT
                    if q + 1 == NPAIR and c == TC - 1:
                        # last pair, last chunk: Wo units whose output
                        # t-tiles only need ots chunks complete after the
                        # first chunk become available now
                        filler = [wo_pending.pop(0)
                                  for _ in range(2 * TC * TC)]
                        nfill = len(filler)
                        spent = 0
                        pair_steps = jmax
                        step = -1  # skip step 1: ots of the prior chunk
                                   # drains on DVE right as this chunk starts
                    otz1 = ps.tile([P, NCHUNK], F32, tag="ot", bufs=2)
                    otz2 = ps.tile([P, NCHUNK], F32, tag="ot", bufs=2)

                    def offn(j):
                        off = max(0, j * P - t0) if causal else 0
                        return off, NCHUNK - off

                    def compute_st(j):
                        # both heads' score tiles in one 2-bank PSUM tile
                        off, n = offn(j)
                        s0 = j * P
                        st12 = ps.tile([P, 2, NCHUNK], F32, tag="st", bufs=2)
                        nc.tensor.matmul(
                            st12[:, 0, :n],
                            kt[0:64, q, s0 : s0 + P],
                            qt[0:64, q, t0 + off : t0 + NCHUNK],
                            start=True, stop=True, tile_position=(0, 0),
                        )
                        nc.tensor.matmul(
                            st12[:, 1, :n],
                            kt[64:128, q, s0 : s0 + P],
                            qt[64:128, q, t0 + off : t0 + NCHUNK],
                            start=True, stop=True, tile_position=(64, 0),
                        )
                        return st12

                    def do_exp(j, st12):
                        # one strided-AP exp evicts both heads' tiles
                        off, n = offn(j)
                        s0 = j * P
                        e12 = sb_e.tile([P, 2, NCHUNK], BF16, tag="e12")
                        nc.scalar.activation(
                            e12[:, :, :n], st12[:, :, :n], Exp, scale=SCALE)
                        if causal and s0 >= t0:
                            # diagonal tile: keep s <= t; one multiplicative
                            # 0/1 mask over both heads (strided AP) on the
                            # otherwise-idle GpSimd: its queue is empty so
                            # the mask starts immediately, unlike DVE whose
                            # FIFO backlog (evictions/z-chain) would delay
                            # the attn@V consumer
                            nc.gpsimd.tensor_mul(
                                e12[:, :, 0:P], e12[:, :, 0:P], tri2_t[:])
                        return e12

                    sts = [compute_st(0)]
                    es = [do_exp(0, sts[0])]
                    if jmax > 1:
                        sts.append(compute_st(1))
                    for j in range(jmax):
                        off, n = offn(j)
                        first, last = (j == 0), (j == jmax - 1)
                        if j + 1 < jmax:
                            es.append(do_exp(j + 1, sts[j + 1]))
                        if j + 2 < jmax:
                            sts.append(compute_st(j + 2))
                        # paced filler: spread this pair's units evenly
                        # across its j-steps (biased one step early so the
                        # last unit lands before the next pair needs it)
                        step += 1
                        while (spent * max(pair_steps - 1, 1) < nfill * step
                               and filler):
                            filler.pop(0)()
                            spent += 1
                        e12 = es[j]
                        # attn @ V fused with Z: stationary [V_h | ones64]
                        # (M=128) -> one e12 stream writes attn@V to rows
                        # 0:64 and the softmax denominator broadcast to
                        # rows 64:128.
                        nc.tensor.matmul(
                            otz1[:, off:], vte_cur[:, j, 0:2, :],
                            e12[:, 0, :n], start=first, stop=last,
                        )
                        nc.tensor.matmul(
                            otz2[:, off:], vte_cur[:, j, 1:3, :],
                            e12[:, 1, :n], start=first, stop=last,
                        )
                    if c == TC - 1:
                        for f in filler:  # leftovers
                            f()
                        filler = []
                    # cross-partition z moves must be plain copies (fused
                    # DVE ISA ops ignore the input partition offset on HW)
                    zs = sb_z.tile([P, NCHUNK], F32, tag="zs")
                    nc.vector.tensor_copy(zs[0:NH, :], otz1[NH:P, :])
                    nc.vector.tensor_copy(zs[NH:P, :], otz2[0:NH, :])
                    zinv = sb_z.tile([P, NCHUNK], F32, tag="zinv")
                    nc.vector.reciprocal_approx_fast(out=zinv[:], in_=zs[:])
                    nc.vector.tensor_mul(
                        ots[0:NH, q, t0 : t0 + NCHUNK],
                        otz1[0:NH, :], zinv[0:NH, :])
                    nc.vector.tensor_mul(
                        ots[NH:P, q, t0 : t0 + NCHUNK],
                        otz2[NH:P, :], zinv[NH:P, :])

            # ---- output projection (tail: units not already interleaved) ----
            for f in wo_pending:
                f()


def build(causal=True, reps=1, no_bias=False):
    nc = bacc.Bacc("TRN2", target_bir_lowering=False, debug=False,
                   num_devices=N_CORES)
    d = {}
    for name in ("xf", "kf", "vf"):
        d[name] = nc.dram_tensor(name, [D, T], BF16, kind="ExternalInput")
    for name in ("wq", "wk", "wv", "wo"):
        d[name] = nc.dram_tensor(name, [D, D], BF16, kind="ExternalInput")
    for name in ("bv", "bo"):
        d[name] = nc.dram_tensor(name, [1, D], BF16, kind="ExternalInput")
    for name in ("bqc", "bkc"):
        d[name] = nc.dram_tensor(name, [P, NPAIR], F32, kind="ExternalInput")
    d["tri2"] = nc.dram_tensor("tri2", [P, 2, P], BF16, kind="ExternalInput")
    d["ones64"] = nc.dram_tensor("ones64", [P, NH], BF16, kind="ExternalInput")
    d["ones1"] = nc.dram_tensor("ones1", [1, P], BF16, kind="ExternalInput")
    d["out"] = nc.dram_tensor("out", [T, D], BF16, kind="ExternalOutput")

    with tile.TileContext(nc) as tc:
        build_body(nc, tc, d, reps=reps, causal=causal, no_bias=no_bias)
    nc.compile()
    return nc


def make_in_maps(input_tensor, keys_vector, values_vector, Wq, bq, Wk, bk,
                 Wv, bv, Wo, bo):
    """Host-side sharding + layout transforms + bf16 casts."""
    def b16(a):
        return np.ascontiguousarray(a).astype(BF16_NP)

    shared = {
        "wq": b16(np.asarray(Wq, np.float32).transpose(1, 0, 2).reshape(D, D)),
        "wk": b16(np.asarray(Wk, np.float32).transpose(1, 0, 2).reshape(D, D)),
        "wv": b16(np.asarray(Wv, np.float32).transpose(1, 0, 2).reshape(D, D)),
        "wo": b16(np.asarray(Wo, np.float32).T),
        # bq/bk as [hn % 128, pair] f32 columns for per-partition bias add
        "bqc": np.ascontiguousarray(
            np.asarray(bq, np.float32).reshape(NPAIR, P).T),
        "bkc": np.ascontiguousarray(
            np.asarray(bk, np.float32).reshape(NPAIR, P).T),
        "bv": b16(np.asarray(bv, np.float32).reshape(1, D)),
        "bo": b16(np.asarray(bo, np.float32).reshape(1, D)),
        "tri2": np.broadcast_to(
            np.triu(np.ones((P, P), np.float32))[:, None, :], (P, 2, P)
        ).astype(BF16_NP).copy(),
        "ones64": np.ones((P, NH), BF16_NP),
        "ones1": np.ones((1, P), BF16_NP),
    }
    x = np.asarray(input_tensor, np.float32)
    k = np.asarray(keys_vector, np.float32)
    v = np.asarray(values_vector, np.float32)
    in_maps = []
    for c in range(N_CORES):
        m = dict(shared)
        m["xf"] = b16(x[:, c, :].T)
        m["kf"] = b16(k[:, c, :].T)
        m["vf"] = b16(v[:, c, :].T)
        in_maps.append(m)
    return in_maps


_NC_CACHE = {}


def kernel(input_tensor, keys_vector, values_vector, Wq, bq, Wk, bk, Wv, bv,
           Wo, bo, mask):
    causal = bool(int(np.asarray(mask)))
    no_bias = all(
        not np.any(np.asarray(b)) for b in (bq, bk, bv, bo)
    )
    key = (causal, no_bias)
    if key not in _NC_CACHE:
        _NC_CACHE[key] = build(causal=causal, no_bias=no_bias)
    nc = _NC_CACHE[key]
    in_maps = make_in_maps(input_tensor, keys_vector, values_vector, Wq, bq,
                           Wk, bk, Wv, bv, Wo, bo)
    res = run_bass_kernel_spmd(nc, in_maps, core_ids=list(range(N_CORES)))
    out = np.empty((T, B, D), np.float32)
    for c in range(N_CORES):
        out[:, c, :] = res.results[c]["out"].astype(np.float32)
    return out
